# revision 11
# baseline (speedup 1.0000x reference)
"""V2: chunk-compacted, load-balanced sharding.

The reference zeroes every timestep t >= lengths[b], so 128-row t-chunks
that are fully masked contribute nothing.  The host drops them during
sharding: the flat list of *live* (batch, chunk) pairs is split evenly
across the 8 cores (chunk granularity, so the per-core byte count is
ceil(total_live_chunks/8) regardless of how ragged the batch is).  Each
core computes a partial [16, 1024] output (masked-exp-weighted sums over
its chunks via PSUM-accumulated matmuls with M=16) and the host sums the
8 partials.

All math stays on device: exp, the softmax normalizer Z (over the full
weight vector), the t < length mask compare, the weighted reduction, and
the final 1/Z scale.  The host only gathers/permutes data (sharding) and
builds index/assignment metadata (tcr: the t-coordinate of each element,
with a +inf sentinel outside the owning row).
"""

import numpy as np

import concourse.bass as bass
import concourse.tile as tile
from concourse import bacc, mybir
from concourse.bass_utils import run_bass_kernel_spmd
from concourse.vector_clock import ScopedClock


class _LeanTileContext(tile.TileContext):
    """TileContext with a lighter kernel epilogue: the trailing
    all-engine barrier after the semaphore clears guards nothing in a
    standalone kernel (no instructions follow), and the sync engine's
    global-clock drain already implies every other engine has finished
    before the clears run."""

    def _drain_and_barrier(self, tick_clock, wait_clock):
        drain_inst = self.nc.sync.drain()
        wait_clock.add_sem_waits(
            drain_inst.ins, ScopedClock({None: tick_clock.global_clock})
        )
        self.nc.all_engine_barrier()
        popped = self.nc._tile_sem_poison_stack.pop()
        assert popped is self._sem_poison
        self.nc.clear_and_free_semaphores(list(self.sems.allocated().values()))

B, T, D = 16, 2048, 1024
NCORES = 8
NCHUNK = T // 128
F32 = mybir.dt.float32
F32R = mybir.dt.float32r

GSZ = 4               # chunks per DMA
USE_F32R = True
XPOOL_BUFS = 4
WARMUP_MMS = 10
FILLER_MMS = 6
BIG = 1.0e9           # t-sentinel for "element not owned by this row"


def _build_program(nchunks, use_f32r=USE_F32R, gsz=GSZ):
    """Build the SPMD program for a per-core chunk count of `nchunks`."""
    nc = bacc.Bacc(
        "TRN2", target_bir_lowering=False, debug=False, num_devices=NCORES
    )
    xdt = F32R if use_f32r else F32
    xc = nc.dram_tensor("xc", [nchunks, 128, D], xdt, kind="ExternalInput").ap()
    w = nc.dram_tensor("w", [1, T], F32, kind="ExternalInput").ap()
    wc = nc.dram_tensor("wc", [128, nchunks], F32, kind="ExternalInput").ap()
    tcr = nc.dram_tensor("tcr", [128, nchunks, B], F32, kind="ExternalInput").ap()
    lens = nc.dram_tensor("lens", [128, B], F32, kind="ExternalInput").ap()
    out = nc.dram_tensor("out", [B, D], F32, kind="ExternalOutput").ap()

    # group sizes: gsz-chunk groups, tapering to single chunks at the end
    sizes = []
    rem = nchunks
    while rem > 0:
        if rem <= 2:
            sizes.append(1); rem -= 1
        else:
            s = min(gsz, rem - 2)
            sizes.append(s); rem -= s
    groups = []
    k0 = 0
    for s in sizes:
        groups.append((k0, s)); k0 += s
    ng = len(groups)

    with _LeanTileContext(nc) as tc:
        with (
            tc.tile_pool(name="consts", bufs=1) as consts,
            tc.tile_pool(name="xin", bufs=1) as xpool,
            tc.tile_pool(name="outs", bufs=1) as opool,
            tc.tile_pool(name="psum", bufs=2, space="PSUM") as pacc,
            tc.tile_pool(name="psumz", bufs=1, space="PSUM") as pz,
        ):
            # --- start the X stream before the (small) metadata loads ---
            from collections import Counter
            tag_counts = Counter(gs for _, gs in groups)
            xts = []
            for g, (k0, gs) in enumerate(groups[:2]):
                xt = xpool.tile([128, gs, D], xdt, name="xt", tag=f"xt{gs}",
                                bufs=tag_counts[gs])
                nc.sync.dma_start(
                    out=xt, in_=xc[k0 : k0 + gs].rearrange("j p d -> p j d"),
                )
                xts.append(xt)

            # --- softmax normalizer Z over the full weight vector ---
            wline = consts.tile([1, T], F32)
            nc.sync.dma_start(out=wline, in_=w)
            eline = consts.tile([1, T], F32)
            zsc = consts.tile([1, 1], F32)
            nc.scalar.activation(
                out=eline, in_=wline, func=mybir.ActivationFunctionType.Exp,
                accum_out=zsc,
            )
            # broadcast Z to 16 partitions via a K=1 matmul, then 1/Z
            ones16 = consts.tile([1, B], F32)
            nc.vector.memset(ones16, 1.0)
            psum_z = pz.tile([B, 1], F32)
            nc.tensor.matmul(psum_z, lhsT=ones16, rhs=zsc, start=True, stop=True)
            rz = consts.tile([B, 1], F32)
            nc.vector.reciprocal(rz, psum_z)

            # --- per-chunk coefficients ---
            wc_sb = consts.tile([128, nchunks], F32)
            nc.sync.dma_start(out=wc_sb, in_=wc)
            ec = consts.tile([128, nchunks], F32)
            nc.scalar.activation(
                out=ec, in_=wc_sb, func=mybir.ActivationFunctionType.Exp,
            )
            tcr_sb = consts.tile([128, nchunks, B], F32)
            nc.sync.dma_start(out=tcr_sb, in_=tcr)
            lens_sb = consts.tile([128, B], F32)
            nc.sync.dma_start(out=lens_sb, in_=lens)

            # mask = (tcr < len_b)  (tcr is BIG outside the owning row)
            lens_b = bass.AP(
                tensor=lens_sb.tensor, offset=lens_sb.offset,
                ap=[lens_sb.ap[0], [0, nchunks], lens_sb.ap[1]],
            )
            c2 = consts.tile([128, nchunks, B], F32R if use_f32r else F32)
            nc.vector.tensor_tensor(
                out=c2, in0=tcr_sb, in1=lens_b, op=mybir.AluOpType.is_lt,
            )
            # c2 *= exp(w) (broadcast over the 16 row slots)
            ec_b = bass.AP(
                tensor=ec.tensor, offset=ec.offset,
                ap=[ec.ap[0], ec.ap[1], [0, B]],
            )
            nc.vector.tensor_tensor(
                out=c2, in0=c2, in1=ec_b, op=mybir.AluOpType.mult,
            )

            # --- PE warm-up: ~4us of dummy back-to-back matmuls lifts the
            # HAM clock gate to 8/8 (2.4 GHz) before the real stream arrives;
            # the steady matmul cadence then keeps it warm. ---
            warm_rhs = consts.tile([128, 512], F32R if use_f32r else F32)
            nc.vector.memset(warm_rhs.bitcast(F32), 0.0)
            warm_lhs = consts.tile([128, 16], F32R if use_f32r else F32)
            nc.vector.memset(warm_lhs.bitcast(F32), 0.0)
            pwarm = pz.tile([16, 512], F32)
            for _ in range(WARMUP_MMS):
                nc.tensor.matmul(pwarm, lhsT=warm_lhs, rhs=warm_rhs,
                                 start=True, stop=True)

            # --- main streaming loop ---
            psf = pacc.tile([B, D], F32, name="psf", tag="ps")
            ps = [psf[:, 0:512], psf[:, 512:1024]]
            for g, (k0, gs) in enumerate(groups):
                if g < 2:
                    xt = xts[g]
                else:
                    xt = xpool.tile([128, gs, D], xdt, name="xt", tag=f"xt{gs}",
                                    bufs=tag_counts[gs])
                    nc.sync.dma_start(
                        out=xt,
                        in_=xc[k0 : k0 + gs].rearrange("j p d -> p j d"),
                    )
                for j in range(gs):
                    k = k0 + j
                    for dh in range(2):
                        nc.tensor.matmul(
                            ps[dh], lhsT=c2[:, k, :],
                            rhs=xt[:, j, dh * 512 : (dh + 1) * 512],
                            start=(k == 0), stop=(k == nchunks - 1),
                        )
                # keep the PE clock warm while waiting for the next transfer
                for _ in range(FILLER_MMS):
                    nc.tensor.matmul(pwarm, lhsT=warm_lhs, rhs=warm_rhs,
                                     start=True, stop=True)

            ot = opool.tile([B, D], F32)
            nc.vector.tensor_scalar(
                out=ot, in0=psf, scalar1=rz, scalar2=None,
                op0=mybir.AluOpType.mult,
            )
            nc.sync.dma_start(out=out, in_=ot)

    nc.compile()
    return nc


_cache = {}


def _get_program(nchunks):
    if nchunks not in _cache:
        _cache[nchunks] = _build_program(nchunks)
    return _cache[nchunks]


def kernel(input, lengths, weights):
    input = np.asarray(input, dtype=np.float32)
    lengths_np = np.asarray(lengths).astype(np.int64)
    weights = np.asarray(weights, dtype=np.float32)

    # --- host-side sharding: pack the exact live (batch, t) rows into
    # 128-row chunks, split evenly across cores (a chunk may mix rows of
    # different batches; the per-partition coefficient metadata carries
    # the (t, batch) identity of every row) ---
    lens_clip = np.clip(lengths_np, 0, T)
    total_rows = int(lens_clip.sum())
    slots = 128 * NCORES
    nchunks = max(1, -(-max(total_rows, 1) // slots))  # chunks per core
    cap = NCORES * nchunks * 128

    b_flat = np.repeat(np.arange(B, dtype=np.int64), lens_clip)
    t_flat = np.concatenate(
        [np.arange(n, dtype=np.int64) for n in lens_clip]
    ) if total_rows else np.zeros(0, dtype=np.int64)
    # pad with dummy rows (batch sentinel -1 -> zero coefficient)
    pad = cap - total_rows
    b_flat = np.concatenate([b_flat, np.full(pad, -1, dtype=np.int64)])
    t_flat = np.concatenate([t_flat, np.zeros(pad, dtype=np.int64)])

    nc = _get_program(nchunks)

    w_np = np.ascontiguousarray(weights)
    lens_rep = np.ascontiguousarray(
        np.broadcast_to(lengths_np.astype(np.float32)[None, :], (128, B))
    )

    flat2d = input.reshape(B * T, D)
    rb = np.arange(B)
    in_maps = []
    for c in range(NCORES):
        sl = slice(c * nchunks * 128, (c + 1) * nchunks * 128)
        bs = b_flat[sl].reshape(nchunks, 128)
        ts = t_flat[sl].reshape(nchunks, 128)

        xc = flat2d[np.maximum(bs, 0) * T + ts]          # [nchunks, 128, D]
        wc = np.ascontiguousarray(weights[ts].T)          # [128, nchunks]
        tcr = np.where(
            bs[:, :, None] == rb[None, None, :],
            ts[:, :, None].astype(np.float32), BIG,
        ).transpose(1, 0, 2)                              # [128, nchunks, B]

        in_maps.append(
            {
                "xc": np.ascontiguousarray(xc),
                "w": w_np.reshape(1, T),
                "wc": wc,
                "tcr": np.ascontiguousarray(tcr.astype(np.float32)),
                "lens": lens_rep,
            }
        )

    res = run_bass_kernel_spmd(nc, in_maps, list(range(NCORES)))
    out = np.zeros((B, D), dtype=np.float32)
    for c in range(NCORES):
        out += res.results[c]["out"]
    return out.astype(np.float32)
